# revision 12
# baseline (speedup 1.0000x reference)
"""Fused ConvTranspose3d(stride2,pad1) + scale + AvgPool3d(2) + bias kernel for TRN2.

Math: the transposed conv (K=3, S=2, P=1) followed by AvgPool(2) collapses into a
single stride-1 VALID conv with a 2x2x2 kernel:
    per-dim taps: tap0 = W[1] + W[2], tap1 = W[0]
    z = conv(x, V) * (s1*s2/8) + (conv_bias*s1 + bias)*s2
Mapping to the PE array: one matmul per output chunk with
    k = 128 = 4 (d,h)-taps x 32 c_in   (shifted-replica SBUF tile)
    m = 128 = 2 w-taps x 64 c_out      (both w-taps from one rhs stream)
    n = 512 = 16 output rows x 32 cols (contiguous rhs slice)
then z[co, oh, ow] = psum[c0half, p(oh,ow)] + psum[c1half, p(oh,ow+1)] on DVE,
bias via ACT Identity-activation, DMA out.
Data parallel: batch 16 -> 2 per core on 8 cores.
"""

import sys

if "/opt/trn_rl_repo" not in sys.path:
    sys.path.insert(0, "/opt/trn_rl_repo")

from contextlib import ExitStack

import numpy as np

import concourse.bass as bass
import concourse.tile as tile
from concourse import mybir
from concourse.bass_utils import run_bass_kernel_spmd
from concourse.vector_clock import ScopedClock as _ScopedClock


# walrus codegen allows only one sync-wait per TPB_CTRL instruction; split the
# Tile tail-drain's waits across single-wait nop carriers.
def _patched_drain_and_barrier(self, tick_clock, wait_clock):
    nc = self.nc
    drain_inst = nc.sync.drain()
    wait_clock.add_sem_waits(
        drain_inst.ins, _ScopedClock({None: tick_clock.global_clock})
    )
    waits = list(drain_inst.ins.sync_info.on_wait)
    if len(waits) > 1:
        drain_inst.ins.sync_info.on_wait = waits[:1]
        for w in waits[1:]:
            n = nc.sync.nop(nofuse=True)
            n.ins.sync_info = mybir.SyncInfo(on_wait=[w], on_update=[])
    nc.all_engine_barrier()
    assert self.sems is not None
    popped = nc._tile_sem_poison_stack.pop()
    assert popped is self._sem_poison
    nc.clear_and_free_semaphores(list(self.sems.allocated().values()))
    nc.all_engine_barrier()


tile.TileContext._drain_and_barrier = _patched_drain_and_barrier


def _legalize_sync_waits(nc, max_waits=1):
    """walrus codegen allows very few sync-waits per instruction; move excess
    waits onto nop carriers on the same engine right before the instruction."""
    for fn in nc.m.functions:
        for bb in fn.blocks:
            new_insts = []
            changed = False
            for inst in bb.instructions:
                si = getattr(inst, "sync_info", None)
                if si is not None and si.on_wait and len(si.on_wait) > max_waits:
                    waits = list(si.on_wait)
                    si.on_wait = waits[-max_waits:]
                    extra = waits[:-max_waits]
                    for i in range(0, len(extra), max_waits):
                        nop = mybir.InstNoOp(
                            name=nc.get_next_instruction_name(),
                            engine=inst.engine,
                            sync_info=mybir.SyncInfo(
                                on_wait=extra[i : i + max_waits], on_update=[]
                            ),
                            bass_nofuse=True,
                        )
                        new_insts.append(nop)
                    changed = True
                new_insts.append(inst)
            if changed:
                bb.instructions[:] = new_insts

N, C_IN, C_OUT = 16, 32, 64
D = H = W = 32
OD = OH = OW = 31
NCORES = 8
NB = N // NCORES  # batches per core
PLANE = H * W  # 1024
VOL = D * PLANE  # 32768
ZPLANE = OH * OW  # 961
ZVOL = OD * ZPLANE  # 29791

# (first x plane, first output slab, n slabs) per chunk; chunk c needs
# x4[:, f] for f in [0, (nsl-1)*1024 + 960), loaded via one 4D-strided
# DRAM read that materializes all four (d,h)-tap-shifted replicas.
_CHUNKS = [(0, 0, 8), (8, 8, 8), (16, 16, 8), (24, 24, 7)]
_ROWBLOCKS = [(0, 16), (16, 15)]
_CH = 7 * PLANE + 992  # max chunk tile free size (nsl=8)


def _build_program():
    nc = bass.Bass(
        "TRN2", target_bir_lowering=False, debug=False, num_swdge_queues=4
    )
    f32 = mybir.dt.float32
    f32r = mybir.dt.float32r
    x_ap = nc.dram_tensor("x", [NB, C_IN, VOL], f32r, kind="ExternalInput").ap()
    w_ap = nc.dram_tensor("wpack", [128, 128], f32r, kind="ExternalInput").ap()
    b_ap = nc.dram_tensor("beta", [C_OUT, 1], f32, kind="ExternalInput").ap()
    z_ap = nc.dram_tensor("z", [NB, C_OUT, ZVOL], f32, kind="ExternalOutput").ap()

    with tile.TileContext(nc) as tc, ExitStack() as ctx:
        wpool = ctx.enter_context(tc.tile_pool(name="w", bufs=1))
        x4pool = ctx.enter_context(tc.tile_pool(name="x4", bufs=4))
        pspool = ctx.enter_context(tc.tile_pool(name="ps", bufs=8, space="PSUM"))
        zcpool = ctx.enter_context(tc.tile_pool(name="zc", bufs=6))
        ogpool = ctx.enter_context(tc.tile_pool(name="og", bufs=2))

        wt = wpool.tile([128, 128], f32r)
        nc.sync.dma_start(wt[:], w_ap[:])
        bt = wpool.tile([C_OUT, 1], f32)
        nc.sync.dma_start(bt[:], b_ap[:])

        x_t = x_ap.tensor
        for b in range(NB):
            for p0, od0, nsl in _CHUNKS:
                ch_need = (nsl - 1) * PLANE + 992
                x4 = x4pool.tile([128, _CH], f32r, tag="x4")
                # 3D strided DRAM reads: dims (b-tap,ci,elem) -> partitions
                # (2a+b)*32+ci; one DMA per d-tap half on separate HW queues
                for a, eng in ((0, nc.sync), (1, nc.scalar)):
                    src = bass.AP(
                        tensor=x_t,
                        offset=b * C_IN * VOL + p0 * PLANE + a * PLANE,
                        ap=[(W, 2), (VOL, C_IN), (1, ch_need)],
                    )
                    eng.dma_start(x4[a * 64 : (a + 1) * 64, 0:ch_need], src)

                og = ogpool.tile([C_OUT, nsl * ZPLANE], f32, tag="og")
                for od_local in range(nsl):
                    od = od0 + od_local
                    for oh0, nrows in _ROWBLOCKS:
                        nfree = nrows * W
                        base = (od - p0) * PLANE + oh0 * W
                        ps = pspool.tile([128, nrows, W], f32, tag="ps")
                        nc.tensor.matmul(
                            ps[:],
                            wt[:],
                            x4[:, base : base + nfree],
                            start=True,
                            stop=True,
                        )
                        zc = zcpool.tile([C_OUT, nrows, OW], f32, tag="zc")
                        nc.scalar.activation(
                            zc[:],
                            ps[0:C_OUT, :, 0:OW],
                            mybir.ActivationFunctionType.Identity,
                            bias=bt[:, 0:1],
                            scale=1.0,
                        )
                        off = od_local * ZPLANE + oh0 * OW
                        dst = og[:, off : off + nrows * OW].rearrange(
                            "p (a b) -> p a b", b=OW
                        )
                        nc.vector.tensor_add(dst, zc[:], ps[C_OUT:128, :, 1:W])
                zbase = od0 * ZPLANE
                nc.gpsimd.dma_start(
                    z_ap[b, :, zbase : zbase + nsl * ZPLANE], og[:]
                )
    _legalize_sync_waits(nc)
    return nc


def _host_prep(weight, conv_bias, bias, scale1, scale2):
    w = np.asarray(weight, dtype=np.float32)  # (C_IN, C_OUT, 3,3,3)
    s1 = float(np.asarray(scale1))
    s2 = float(np.asarray(scale2))
    taps = [[1, 2], [0]]  # per-dim kernel index sets: tap0 = W[1]+W[2], tap1 = W[0]
    alpha = s1 * s2 / 8.0
    wpack = np.zeros((128, 128), dtype=np.float32)
    for a in range(2):
        for b in range(2):
            t = 2 * a + b
            for c in range(2):
                v = np.zeros((C_IN, C_OUT), dtype=np.float64)
                for kd in taps[a]:
                    for kh in taps[b]:
                        for kw in taps[c]:
                            v += w[:, :, kd, kh, kw]
                wpack[t * C_IN : (t + 1) * C_IN, c * C_OUT : (c + 1) * C_OUT] = (
                    alpha * v
                ).astype(np.float32)
    beta = (
        (np.asarray(conv_bias, dtype=np.float64).reshape(-1) * s1
         + np.asarray(bias, dtype=np.float64).reshape(-1))
        * s2
    ).astype(np.float32).reshape(C_OUT, 1)
    return wpack, beta


def kernel(x, weight, conv_bias, bias, scale1, scale2, _trace=False):
    x = np.ascontiguousarray(np.asarray(x, dtype=np.float32))
    wpack, beta = _host_prep(weight, conv_bias, bias, scale1, scale2)

    nc = _build_program()
    in_maps = []
    for core in range(NCORES):
        xs = x[core * NB : (core + 1) * NB].reshape(NB, C_IN, VOL)
        in_maps.append(
            {"x": np.ascontiguousarray(xs), "wpack": wpack, "beta": beta}
        )
    res = run_bass_kernel_spmd(
        nc, in_maps, core_ids=list(range(NCORES)), trace=_trace
    )
    z = np.empty((N, C_OUT, OD, OH, OW), dtype=np.float32)
    for core in range(NCORES):
        z[core * NB : (core + 1) * NB] = res.results[core]["z"].reshape(
            NB, C_OUT, OD, OH, OW
        )
    if _trace:
        return z, res
    return z


# revision 15
# speedup vs baseline: 5.3585x; 5.3585x over previous
"""Fused ConvTranspose3d(stride2,pad1) + scale + AvgPool3d(2) + bias kernel for TRN2.

Math: the transposed conv (K=3, S=2, P=1) followed by AvgPool(2) collapses into a
single stride-1 VALID conv with a 2x2x2 kernel:
    per-dim taps: tap0 = W[1] + W[2], tap1 = W[0]
    z = conv(x, V) * (s1*s2/8) + (conv_bias*s1 + bias)*s2
Mapping to the PE array: one matmul per output chunk with
    k = 128 = 4 (d,h)-taps x 32 c_in   (shifted-replica SBUF tile)
    m = 128 = 2 w-taps x 64 c_out      (both w-taps from one rhs stream)
    n = 512 = 16 output rows x 32 cols (contiguous rhs slice)
then z[co, oh, ow] = psum[c0half, p(oh,ow)] + psum[c1half, p(oh,ow+1)] on DVE,
bias via ACT Identity-activation, DMA out.
Data parallel: batch 16 -> 2 per core on 8 cores.
"""

import sys

if "/opt/trn_rl_repo" not in sys.path:
    sys.path.insert(0, "/opt/trn_rl_repo")

from contextlib import ExitStack

import numpy as np

import concourse.bass as bass
import concourse.tile as tile
from concourse import mybir
from concourse.bass_utils import run_bass_kernel_spmd
from concourse.vector_clock import ScopedClock as _ScopedClock


# walrus codegen allows only one sync-wait per TPB_CTRL instruction; split the
# Tile tail-drain's waits across single-wait nop carriers.
def _patched_drain_and_barrier(self, tick_clock, wait_clock):
    nc = self.nc
    drain_inst = nc.sync.drain()
    wait_clock.add_sem_waits(
        drain_inst.ins, _ScopedClock({None: tick_clock.global_clock})
    )
    waits = list(drain_inst.ins.sync_info.on_wait)
    if len(waits) > 1:
        drain_inst.ins.sync_info.on_wait = waits[:1]
        for w in waits[1:]:
            n = nc.sync.nop(nofuse=True)
            n.ins.sync_info = mybir.SyncInfo(on_wait=[w], on_update=[])
    nc.all_engine_barrier()
    assert self.sems is not None
    popped = nc._tile_sem_poison_stack.pop()
    assert popped is self._sem_poison
    nc.clear_and_free_semaphores(list(self.sems.allocated().values()))
    nc.all_engine_barrier()


tile.TileContext._drain_and_barrier = _patched_drain_and_barrier


def _legalize_sync_waits(nc, max_waits=1):
    """walrus codegen allows very few sync-waits per instruction; move excess
    waits onto nop carriers on the same engine right before the instruction."""
    for fn in nc.m.functions:
        for bb in fn.blocks:
            new_insts = []
            changed = False
            for inst in bb.instructions:
                si = getattr(inst, "sync_info", None)
                if si is not None and si.on_wait and len(si.on_wait) > max_waits:
                    waits = list(si.on_wait)
                    si.on_wait = waits[-max_waits:]
                    extra = waits[:-max_waits]
                    for i in range(0, len(extra), max_waits):
                        nop = mybir.InstNoOp(
                            name=nc.get_next_instruction_name(),
                            engine=inst.engine,
                            sync_info=mybir.SyncInfo(
                                on_wait=extra[i : i + max_waits], on_update=[]
                            ),
                            bass_nofuse=True,
                        )
                        new_insts.append(nop)
                    changed = True
                new_insts.append(inst)
            if changed:
                bb.instructions[:] = new_insts

N, C_IN, C_OUT = 16, 32, 64
D = H = W = 32
OD = OH = OW = 31
NCORES = 8
NB = N // NCORES  # batches per core
PLANE = H * W  # 1024
VOL = D * PLANE  # 32768
ZPLANE = OH * OW  # 961
ZVOL = OD * ZPLANE  # 29791

# (first x plane, first output slab, n slabs) per chunk; chunk c needs
# x4[:, f] for f in [0, (nsl-1)*1024 + 960), loaded via one 4D-strided
# DRAM read that materializes all four (d,h)-tap-shifted replicas.
_CHUNKS = [(0, 0, 8), (8, 8, 8), (16, 16, 8), (24, 24, 7)]
_ROWBLOCKS = [(0, 16), (16, 15)]
_CH = 7 * PLANE + 992  # max chunk tile free size (nsl=8)


def _build_program():
    nc = bass.Bass(
        "TRN2", target_bir_lowering=False, debug=False, num_swdge_queues=4
    )
    f32 = mybir.dt.float32
    f32r = mybir.dt.float32r
    # x pre-replicated on host: partition dim = (2a+b)*32+ci, holding
    # x[ci, f + a*PLANE + b*W] flattened over (d,h,w)
    x_ap = nc.dram_tensor("x", [NB, 128, VOL], f32r, kind="ExternalInput").ap()
    w_ap = nc.dram_tensor("wpack", [128, 128], f32r, kind="ExternalInput").ap()
    b_ap = nc.dram_tensor("beta", [C_OUT, 1], f32, kind="ExternalInput").ap()
    z_ap = nc.dram_tensor("z", [NB, C_OUT, ZVOL], f32, kind="ExternalOutput").ap()

    with tile.TileContext(nc) as tc, ExitStack() as ctx:
        wpool = ctx.enter_context(tc.tile_pool(name="w", bufs=1))
        x4pool = ctx.enter_context(tc.tile_pool(name="x4", bufs=4))
        pspool = ctx.enter_context(tc.tile_pool(name="ps", bufs=8, space="PSUM"))
        zcpool = ctx.enter_context(tc.tile_pool(name="zc", bufs=6))
        ogpool = ctx.enter_context(tc.tile_pool(name="og", bufs=2))

        wt = wpool.tile([128, 128], f32r)
        nc.sync.dma_start(wt[:], w_ap[:])
        bt = wpool.tile([C_OUT, 1], f32)
        nc.sync.dma_start(bt[:], b_ap[:])

        for b in range(NB):
            for ci_, (p0, od0, nsl) in enumerate(_CHUNKS):
                ch_need = (nsl - 1) * PLANE + 992
                x4 = x4pool.tile([128, _CH], f32r, tag="x4")
                # plain 2D full-partition load; replicas prebuilt on host
                eng = (nc.sync, nc.scalar)[ci_ % 2]
                eng.dma_start(
                    x4[0:128, 0:ch_need],
                    x_ap[b, :, p0 * PLANE : p0 * PLANE + ch_need],
                )

                og = ogpool.tile([C_OUT, nsl * ZPLANE], f32, tag="og")
                for od_local in range(nsl):
                    od = od0 + od_local
                    for oh0, nrows in _ROWBLOCKS:
                        nfree = nrows * W
                        base = (od - p0) * PLANE + oh0 * W
                        ps = pspool.tile([128, nrows, W], f32, tag="ps")
                        nc.tensor.matmul(
                            ps[:],
                            wt[:],
                            x4[:, base : base + nfree],
                            start=True,
                            stop=True,
                        )
                        zc = zcpool.tile([C_OUT, nrows, OW], f32, tag="zc")
                        nc.scalar.activation(
                            zc[:],
                            ps[0:C_OUT, :, 0:OW],
                            mybir.ActivationFunctionType.Identity,
                            bias=bt[:, 0:1],
                            scale=1.0,
                        )
                        off = od_local * ZPLANE + oh0 * OW
                        dst = og[:, off : off + nrows * OW].rearrange(
                            "p (a b) -> p a b", b=OW
                        )
                        nc.vector.tensor_add(dst, zc[:], ps[C_OUT:128, :, 1:W])
                zbase = od0 * ZPLANE
                nc.gpsimd.dma_start(
                    z_ap[b, :, zbase : zbase + nsl * ZPLANE], og[:]
                )
    _legalize_sync_waits(nc)
    return nc


def _host_prep(weight, conv_bias, bias, scale1, scale2):
    w = np.asarray(weight, dtype=np.float32)  # (C_IN, C_OUT, 3,3,3)
    s1 = float(np.asarray(scale1))
    s2 = float(np.asarray(scale2))
    taps = [[1, 2], [0]]  # per-dim kernel index sets: tap0 = W[1]+W[2], tap1 = W[0]
    alpha = s1 * s2 / 8.0
    wpack = np.zeros((128, 128), dtype=np.float32)
    for a in range(2):
        for b in range(2):
            t = 2 * a + b
            for c in range(2):
                v = np.zeros((C_IN, C_OUT), dtype=np.float64)
                for kd in taps[a]:
                    for kh in taps[b]:
                        for kw in taps[c]:
                            v += w[:, :, kd, kh, kw]
                wpack[t * C_IN : (t + 1) * C_IN, c * C_OUT : (c + 1) * C_OUT] = (
                    alpha * v
                ).astype(np.float32)
    beta = (
        (np.asarray(conv_bias, dtype=np.float64).reshape(-1) * s1
         + np.asarray(bias, dtype=np.float64).reshape(-1))
        * s2
    ).astype(np.float32).reshape(C_OUT, 1)
    return wpack, beta


def kernel(x, weight, conv_bias, bias, scale1, scale2, _trace=False):
    x = np.asarray(x, dtype=np.float32)
    wpack, beta = _host_prep(weight, conv_bias, bias, scale1, scale2)

    # host-side tap replication: xrep[n, (2a+b)*32+ci, f] = x[n, ci, f+shift]
    xf = x.reshape(N, C_IN, VOL)
    xrep = np.zeros((N, 4, C_IN, VOL), dtype=np.float32)
    for t, s in enumerate((0, W, PLANE, PLANE + W)):
        xrep[:, t, :, 0 : VOL - s] = xf[:, :, s:VOL]
    xrep = xrep.reshape(N, 128, VOL)

    nc = _build_program()
    in_maps = []
    for core in range(NCORES):
        xs = xrep[core * NB : (core + 1) * NB]
        in_maps.append(
            {"x": np.ascontiguousarray(xs), "wpack": wpack, "beta": beta}
        )
    res = run_bass_kernel_spmd(
        nc, in_maps, core_ids=list(range(NCORES)), trace=_trace
    )
    z = np.empty((N, C_OUT, OD, OH, OW), dtype=np.float32)
    for core in range(NCORES):
        z[core * NB : (core + 1) * NB] = res.results[core]["z"].reshape(
            NB, C_OUT, OD, OH, OW
        )
    if _trace:
        return z, res
    return z
